# revision 41
# baseline (speedup 1.0000x reference)
"""Bidirectional simplified SSM kernel for Trainium2 (8 NeuronCores).

Math (per batch element b):
    z = x @ W_in                                  [L, DI]
    fwd:  o = z @ W_fwd; delta = sigmoid(o[:, :DI]); gate = o[:, DI:] * z
          h_t = delta_t * h_{t-1} + gate_t        (t ascending)
    bwd:  same with W_bwd, t descending
    y    = concat(h_fwd, h_bwd) @ W_out + x
    out  = LayerNorm(y) * gamma + beta

Sharding: 8 cores = 4 batches x 2 sequence halves, each with a 64-token
halo (delta ~ sigmoid(small) ~ 0.5 forgets cross-boundary state to
~1e-19 over 64 steps; no cross-core communication).

Precision/layout plan (rel err ~1.3e-2 vs the 2e-2 gate):
 - Host ships x twice: natural fp16 (residual/LN) and pre-transposed
   fp8 (z GEMM rhs), plus weights pre-packed in exact SBUF layout.
 - z GEMM: fp8 DoubleRow (2 K-tiles/instr, 0.5 cycles/row) with a
   split-W_in correction pass (W_in8 + fp8(W_in - W_in8)).
 - o GEMM: fp16 over the fp16 z (kills the z8/W quantization terms).
 - out GEMM: fp8 DoubleRow over the fp8 scan output h, with a split
   W_out correction pass.
 - Row sums of x for the LayerNorm mean ship precomputed from the host.

Engine plan: PE z/o/out GEMMs; ACT sigmoids + PSUM->SBUF converts +
copy-with-rowsum + half the squares + sqrt; DVE gates, all four scan
chains, residual add (fp16 2x), the other squares via mult + running-
sum scan, normalize (fp16 4x tensor_scalar); GPSIMD stats smalls and
most normalizes.  All input DMAs issue from SP HWDGE in priority order
(the transposed x streams through a 2-buffer rotation); y chunks DMA
out per-chunk as their normalize completes, middle-out, software-
pipelined with a 3-stage lag so no queue head-of-line blocks.

Hardware-validity notes learned the hard way: GPSIMD cannot access
PSUM or run scan/stt/divide ops; tensor_tensor_reduce crashes the
runtime; PSUM-draining reads must cover whole accumulation groups.
"""

import os
import sys

for _p in ("/opt/trn_rl_repo", "/root/.axon_site/_ro/trn_rl_repo"):
    if os.path.isdir(_p) and _p not in sys.path:
        sys.path.insert(0, _p)

import ml_dtypes
import numpy as np

import concourse.bacc as bacc
import concourse.bass as bass
import concourse.mybir as mybir
import concourse.tile as tile

P = 128
LN_EPS = 1e-5

B, L, D, DI = 4, 4096, 2048, 256
HALO = 64
T_CORE = L // 2            # tokens owned per core
T_CTX = T_CORE + 2 * HALO  # context tokens incl. halo
T_SCAN = T_CORE + HALO     # tokens each direction scans over
N_CORES = 8

F8 = ml_dtypes.float8_e4m3
DR = mybir.MatmulPerfMode.DoubleRow

# interleaved so both scan directions get their first segment early
SEG_ORDER = [0, 4, 1, 3, 2]
# middle-out: middle chunks' h_fwd/h_bwd complete first
CHUNK_ORDER = [9, 10, 8, 11, 7, 12, 6, 13, 5, 14, 4, 15, 3, 2, 1, 0]


def build_nc():
    d, di = D, DI
    kd = d // P            # 16 K-blocks for the z GEMM
    ki = di // P           # 2  channel groups of DI
    mi2 = 2 * di // P      # 4  output channel groups of the o GEMM
    ncho = T_CORE // P     # 16 owned output chunks
    segs = [(s, min(512, T_CTX - s)) for s in range(0, T_CTX, 512)]
    ssegs = [(s, min(512, T_SCAN - s)) for s in range(0, T_SCAN, 512)]
    nseg = len(segs)
    assert nseg == len(ssegs) == len(SEG_ORDER)

    f8 = mybir.dt.float8e4
    f16 = mybir.dt.float16
    f32 = mybir.dt.float32
    AO = mybir.AluOpType
    AF = mybir.ActivationFunctionType

    nc = bacc.Bacc("TRN2", target_bir_lowering=False, debug=False)
    xth_d = nc.dram_tensor("xT8h", [P, kd, T_CTX], f8, kind="ExternalInput").ap()
    x_d = nc.dram_tensor("x16", [T_CORE, d], f16, kind="ExternalInput").ap()
    win_d = nc.dram_tensor("W_in8", [P, kd, di], f8, kind="ExternalInput").ap()
    winr_d = nc.dram_tensor("W_in8r", [P, kd, di], f8, kind="ExternalInput").ap()
    wf_d = nc.dram_tensor("W_fwd16", [P, ki, 2 * di], f16, kind="ExternalInput").ap()
    wb_d = nc.dram_tensor("W_bwd16", [P, ki, 2 * di], f16, kind="ExternalInput").ap()
    wo_d = nc.dram_tensor("W_out8", [P, mi2, d], f8, kind="ExternalInput").ap()
    wor_d = nc.dram_tensor("W_out8r", [P, mi2, d], f8, kind="ExternalInput").ap()
    sx_d = nc.dram_tensor("sx", [P, T_CORE // P], f32, kind="ExternalInput").ap()
    y_d = nc.dram_tensor("y", [T_CORE, d], f16, kind="ExternalOutput").ap()

    inv_d = 1.0 / d

    with tile.TileContext(nc) as tc:
        with (
            tc.tile_pool(name="const", bufs=1) as cpool,
            tc.tile_pool(name="xt", bufs=1) as xtpool,
            tc.tile_pool(name="xn", bufs=1) as xnpool,
            tc.tile_pool(name="z", bufs=1) as zpool,
            tc.tile_pool(name="dg", bufs=1) as dgpool,
            tc.tile_pool(name="y16", bufs=3) as ypool,
            tc.tile_pool(name="ssm", bufs=2) as spool,
            tc.tile_pool(name="sc", bufs=1) as scpool,
            tc.tile_pool(name="sq", bufs=1) as sqpool,
            tc.tile_pool(name="yo", bufs=2) as yopool,
            tc.tile_pool(name="st", bufs=4) as stpool,
            tc.tile_pool(name="ps", bufs=4, space="PSUM") as pspool,
        ):
            # ---- pool-issued input DMAs, priority order ----
            w_in8 = cpool.tile([P, kd, di], f8)
            w_in8r = cpool.tile([P, kd, di], f8)
            w_f16 = cpool.tile([P, ki, 2 * di], f16)
            w_b16 = cpool.tile([P, ki, 2 * di], f16)
            w_o8 = cpool.tile([P, mi2, d], f8)
            w_o8r = cpool.tile([P, mi2, d], f8)
            x16 = xnpool.tile([P, ncho, d], f16)
            # x^T streams through a 2-buffer rotation (not resident)
            xth = [xtpool.tile([P, kd, 512], f8, name=f"xth{i}")
                   for i in range(2)]

            eps_t = cpool.tile([P, 1], f32)
            nc.gpsimd.memset(eps_t[:], LN_EPS)
            sx = cpool.tile([P, ncho], f32)
            nc.sync.dma_start(sx[:], sx_d)
            ones16 = cpool.tile([P, d], f16)
            nc.gpsimd.memset(ones16[:], 1.0)

            # all input DMAs on SP HWDGE (SEQ frees before the transfer, and
            # the pool queue stays clear for gate/scan work); transfer order
            # on the DMA engines = issue order = priority order
            def x16_quad(q):
                nc.sync.dma_start(
                    x16[:, 4 * q:4 * q + 4, :],
                    x_d[512 * q:512 * (q + 1), :].rearrange(
                        "(c p) d -> p c d", p=P
                    ),
                )

            def xt8_seg(k):
                si = SEG_ORDER[k]
                s0, ssz = segs[si]
                nc.sync.dma_start(
                    xth[k % 2][:, :, :ssz], xth_d[:, :, s0:s0 + ssz]
                )

            # only xth[1]'s tail is ever read beyond its DMA'd width (the
            # 128-token segment 4 reads the full 512); disjoint from the DMA
            # region so the transfer is not delayed
            nc.gpsimd.memset(xth[1][:, :, 128:], 0.0)
            nc.sync.dma_start(w_in8[:], win_d)
            nc.sync.dma_start(w_in8r[:], winr_d)
            xt8_seg(0)
            xt8_seg(1)
            nc.sync.dma_start(w_f16[:], wf_d)
            nc.sync.dma_start(w_b16[:], wb_d)


            # ---- z GEMM (double-fp8 DoubleRow) + o GEMMs, seg-interleaved ----
            # widths padded so every PSUM-draining op covers the full 512
            # columns of its PSUM tile (partial reads would leave a WAR gap
            # against the next accumulation group on the same bank)
            zw = (T_CTX // 512 + 1) * 512 + HALO          # 2624
            dgw = (T_SCAN // 512 + 1) * 512               # 2560
            z16 = zpool.tile([P, ki, zw], f16)
            d_f = dgpool.tile([P, ki, dgw], f16)
            g_f = dgpool.tile([P, ki, dgw], f16)
            h_f = dgpool.tile([P, ki, T_SCAN], f8)
            d_b = dgpool.tile([P, ki, dgw], f16)
            g_b = dgpool.tile([P, ki, dgw], f16)
            h_b = dgpool.tile([P, ki, T_SCAN], f8)

            def z_seg(k):
                si = SEG_ORDER[k]
                s0, ssz = segs[si]
                xh = xth[k % 2]
                pz = pspool.tile([P, 1024], f32, tag="ps", name="pz")
                for m in range(ki):
                    pv = pz[:, m * 512:(m + 1) * 512]
                    passes = [(w_in8, xh), (w_in8r, xh)]
                    for pi, (w8, xs) in enumerate(passes):
                        for k8 in range(kd // 2):
                            nc.tensor.matmul(
                                pv,
                                w8[:, 2 * k8:2 * k8 + 2, m * P:(m + 1) * P],
                                xs[:, 2 * k8:2 * k8 + 2, :],
                                start=(pi == 0 and k8 == 0),
                                stop=(pi == 1 and k8 == kd // 2 - 1),
                                perf_mode=DR,
                            )
                # full-width 2D convert: depends on both accumulation groups
                nc.scalar.copy(z16[:, :, s0:s0 + 512], pz[:])

            def o_seg(si, reverse):
                s0, ssz = ssegs[si]
                tok_off = HALO if reverse else 0
                w16 = w_b16 if reverse else w_f16
                dt = d_b if reverse else d_f
                gt = g_b if reverse else g_f
                zsl = slice(tok_off + s0, tok_off + s0 + 512)
                # deltas and gates in separate PSUM tiles: the ACT sigmoids
                # drain poA fast; only poB waits on the gate engines
                poA = pspool.tile([P, 1024], f32, tag="ps", name="poA")
                poB = pspool.tile([P, 1024], f32, tag="ps", name="poB")
                for m2 in range(mi2):
                    po = poA if m2 < ki else poB
                    pv = po[:, (m2 % ki) * 512:(m2 % ki + 1) * 512]
                    for kb in range(ki):
                        nc.tensor.matmul(
                            pv,
                            w16[:, kb, m2 * P:(m2 + 1) * P],
                            z16[:, kb, zsl],
                            start=(kb == 0),
                            stop=(kb == ki - 1),
                        )
                # GPSIMD cannot touch PSUM, so sigmoids (ACT) and gates
                # (DVE) drain it fused with their real work, one 3D
                # instruction per segment each
                nc.scalar.activation(
                    dt[:, :, s0:s0 + 512], poA[:], AF.Sigmoid
                )
                nc.vector.tensor_tensor(
                    gt[:, :, s0:s0 + 512], poB[:],
                    z16[:, :, zsl], AO.mult,
                )

            def scan_seg(si, reverse):
                s0, ssz = ssegs[si]
                dt, gt, ht = (d_b, g_b, h_b) if reverse else (d_f, g_f, h_f)
                first = si == (len(ssegs) - 1 if reverse else 0)
                for kb in range(ki):
                    e = nc.vector
                    if not reverse:
                        init = 0.0 if first else ht[:, kb, s0 - 1:s0]
                        e.tensor_tensor_scan(
                            ht[:, kb, s0:s0 + ssz],
                            dt[:, kb, s0:s0 + ssz],
                            gt[:, kb, s0:s0 + ssz],
                            init,
                            AO.mult,
                            AO.add,
                        )
                    else:
                        hi = s0 + ssz
                        init = 0.0 if first else ht[:, kb, hi:hi + 1]
                        e.tensor_tensor_scan(
                            ht[:, kb, s0:s0 + ssz][:, ::-1],
                            dt[:, kb, s0:s0 + ssz][:, ::-1],
                            gt[:, kb, s0:s0 + ssz][:, ::-1],
                            init,
                            AO.mult,
                            AO.add,
                        )

            # PE/consumer order: z segs interleaved with o segs as the
            # transposed input lands; fwd o ascending, bwd o descending.
            # pad region read by the last bwd o-segment, never written
            nc.gpsimd.memset(z16[:, :, 5 * 512:], 0.0)
            z_seg(0)
            xt8_seg(2)
            z_seg(1)
            xt8_seg(3)
            o_seg(0, reverse=False)
            o_seg(nseg - 1, reverse=True)
            scan_seg(0, reverse=False)
            scan_seg(nseg - 1, reverse=True)
            fwd_i, bwd_i = 1, nseg - 2
            for k in range(2, nseg):
                z_seg(k)
                if k == 2:
                    xt8_seg(4)
                    x16_quad(2)
                    nc.sync.dma_start(w_o8[:], wo_d)
                if k == 3:
                    x16_quad(3)
                    nc.sync.dma_start(w_o8r[:], wor_d)
                if k == 4:
                    x16_quad(1)
                    x16_quad(0)
                if k % 2 == 0:
                    o_seg(fwd_i, reverse=False)
                    scan_seg(fwd_i, reverse=False)
                    fwd_i += 1
                else:
                    o_seg(bwd_i, reverse=True)
                    scan_seg(bwd_i, reverse=True)
                    bwd_i -= 1
            while fwd_i < nseg or bwd_i >= 0:
                if fwd_i < nseg:
                    o_seg(fwd_i, reverse=False)
                    scan_seg(fwd_i, reverse=False)
                    fwd_i += 1
                if bwd_i >= 0:
                    o_seg(bwd_i, reverse=True)
                    scan_seg(bwd_i, reverse=True)
                    bwd_i -= 1

            # ---- out GEMM + residual + LayerNorm per owned chunk ----
            # Four emission stages with 1-chunk lags so the in-order queues
            # never head-of-line block on the cross-engine stat chain.
            # tensor_tensor_reduce is broken in the HW runtime, so:
            #   A: PE out GEMM (fp8 DR, W_out hi+lo); ACT copy+accum
            #      (ssm16 + row-sum of the ssm part; sum(x) ships from host)
            #   B: DVE residual add (fp16 2x); sumsq via ACT Square+accum on
            #      ~half the chunks, else DVE mult + running-sum scan;
            #      GPSIMD mean stats
            #   C: GPSIMD var; ACT sqrt; DVE reciprocal
            #   D: normalize (DVE/GPSIMD tensor_scalar) + SP y DMA
            live = {}

            def chunk_a(oc):
                tb = HALO + oc * P     # context-token base of this chunk
                ssm16 = spool.tile([P, d], f16, name="ssm16")
                st = stpool.tile([P, 12], f32, name="st")
                pys = [pspool.tile([P, 1024], f32, tag="ps", name="py")
                       for _ in range(2)]
                for dgi in range(4):
                    dsl = slice(dgi * 512, (dgi + 1) * 512)
                    pv = pys[dgi // 2][:, (dgi % 2) * 512:(dgi % 2 + 1) * 512]
                    nc.tensor.matmul(
                        pv, h_f[:, :, tb:tb + P], w_o8[:, 0:2, dsl],
                        start=True, stop=False, perf_mode=DR,
                    )
                    nc.tensor.matmul(
                        pv, h_b[:, :, tb - HALO:tb - HALO + P],
                        w_o8[:, 2:4, dsl],
                        start=False, stop=False, perf_mode=DR,
                    )
                    nc.tensor.matmul(
                        pv, h_f[:, :, tb:tb + P], w_o8r[:, 0:2, dsl],
                        start=False, stop=False, perf_mode=DR,
                    )
                    nc.tensor.matmul(
                        pv, h_b[:, :, tb - HALO:tb - HALO + P],
                        w_o8r[:, 2:4, dsl],
                        start=False, stop=True, perf_mode=DR,
                    )
                for half in range(2):
                    hsl = slice(half * 1024, (half + 1) * 1024)
                    nc.scalar.activation(
                        ssm16[:, hsl], pys[half][:], AF.Copy,
                        accum_out=st[:, half:half + 1],
                    )
                live[oc] = (ssm16, st)

            def chunk_b(oc, idx):
                ssm16, st = live[oc]
                y16 = ypool.tile([P, d], f16, name="y16")
                nc.vector.tensor_tensor(
                    y16[:], ssm16[:], x16[:, oc, :], AO.add
                )
                if idx % 2 == 0:
                    # sumsq on DVE: square then running-sum scan; the last
                    # column is sum(y^2)
                    sq = sqpool.tile([P, d], f16, name="sq")
                    nc.vector.tensor_tensor(sq[:], y16[:], y16[:], AO.mult)
                    so = scpool.tile([P, d], f16, name="so")
                    nc.vector.tensor_tensor_scan(
                        so[:], ones16[:], sq[:], 0.0, AO.mult, AO.add
                    )
                    live[oc] = (y16, st, so[:, d - 1:d])
                else:
                    sq = sqpool.tile([P, d], f16, name="sq")
                    nc.scalar.activation(
                        sq[:], y16[:], AF.Square, accum_out=st[:, 2:3]
                    )
                    live[oc] = (y16, st, st[:, 2:3])
                # mean = (st0+st1+sum_x)/d
                nc.gpsimd.tensor_tensor(st[:, 3:4], st[:, 0:1], st[:, 1:2], AO.add)
                nc.gpsimd.tensor_tensor(
                    st[:, 10:11], st[:, 3:4], sx[:, oc:oc + 1], AO.add
                )
                nc.gpsimd.tensor_scalar(st[:, 4:5], st[:, 10:11], inv_d, None, AO.mult)
                nc.gpsimd.tensor_tensor(st[:, 5:6], st[:, 4:5], st[:, 4:5], AO.mult)

            def chunk_c(oc):
                y16, st, sumsq = live[oc]
                nc.gpsimd.tensor_scalar(st[:, 9:10], sumsq, inv_d, None, AO.mult)
                nc.gpsimd.tensor_tensor(st[:, 6:7], st[:, 9:10], st[:, 5:6], AO.subtract)
                nc.scalar.activation(st[:, 7:8], st[:, 6:7], AF.Sqrt, bias=eps_t[:])
                nc.vector.reciprocal(st[:, 8:9], st[:, 7:8])

            def chunk_d(oc, idx):
                y16, st, _ = live.pop(oc)
                yo = yopool.tile([P, d], f16, name="yo")
                # normalize: GPSIMD (idle after the scans) takes most chunks
                e = nc.gpsimd if idx % 8 < 5 else nc.vector
                e.tensor_scalar(
                    yo[:], y16[:], st[:, 4:5], st[:, 8:9], AO.subtract, AO.mult
                )
                nc.sync.dma_start(y_d[oc * P:(oc + 1) * P, :], yo[:])

            for idx in range(ncho + 3):
                if idx < ncho:
                    chunk_a(CHUNK_ORDER[idx])
                if 1 <= idx < ncho + 1:
                    chunk_b(CHUNK_ORDER[idx - 1], idx - 1)
                if 2 <= idx < ncho + 2:
                    chunk_c(CHUNK_ORDER[idx - 2])
                if idx >= 3:
                    chunk_d(CHUNK_ORDER[idx - 3], idx - 3)

    nc.compile()
    return nc


_NC_CACHE = {}


def _get_nc():
    if "nc" not in _NC_CACHE:
        _NC_CACHE["nc"] = build_nc()
    return _NC_CACHE["nc"]


def _pack_weights(W_in, W_fwd, W_bwd, W_out):
    """Rearrange [K, M] weights into SBUF layout [128, K//128, M]."""
    def pack(w, dt):
        k, m = w.shape
        return np.ascontiguousarray(
            w.reshape(k // P, P, m).transpose(1, 0, 2)
        ).astype(dt)

    W_in = np.asarray(W_in, np.float32)
    W_in8 = W_in.astype(F8)
    W_in8r = (W_in - W_in8.astype(np.float32)).astype(F8)
    W_out = np.asarray(W_out, np.float32)
    W_out8 = W_out.astype(F8)
    W_out8r = (W_out - W_out8.astype(np.float32)).astype(F8)
    return {
        "W_in8": pack(W_in8.astype(np.float32), F8),
        "W_in8r": pack(W_in8r.astype(np.float32), F8),
        "W_fwd16": pack(np.asarray(W_fwd, np.float32), np.float16),
        "W_bwd16": pack(np.asarray(W_bwd, np.float32), np.float16),
        "W_out8": pack(W_out8.astype(np.float32), F8),
        "W_out8r": pack(W_out8r.astype(np.float32), F8),
    }


def shard_inputs(x, W_in, W_fwd, W_bwd, W_out):
    """Full x [B, L, D] -> 8 per-core input dicts."""
    x16 = np.asarray(x, np.float32).astype(np.float16)
    xpad = np.zeros((B, L + 2 * HALO, D), np.float16)
    xpad[:, HALO:HALO + L] = x16
    wmaps = _pack_weights(W_in, W_fwd, W_bwd, W_out)
    in_maps = []
    for b in range(B):
        for h in range(2):
            ctx = xpad[b, h * T_CORE:h * T_CORE + T_CTX]      # [T_CTX, D]
            xT = np.ascontiguousarray(
                ctx.T.reshape(D // P, P, T_CTX).transpose(1, 0, 2)
            )                                                  # [128, kd, T_CTX]
            xT8h = xT.astype(F8)
            xnat = np.ascontiguousarray(ctx[HALO:HALO + T_CORE])
            sx = np.ascontiguousarray(
                xnat.astype(np.float32).sum(axis=1).reshape(T_CORE // P, P).T
            )
            in_maps.append({"xT8h": xT8h, "x16": xnat, "sx": sx, **wmaps})
    return in_maps


def gather_outputs(results):
    out = np.empty((B, L, D), np.float32)
    for b in range(B):
        for h in range(2):
            out[b, h * T_CORE:(h + 1) * T_CORE] = results[b * 2 + h]["y"]
    return out


def run_on_hw(x, W_in, W_fwd, W_bwd, W_out, trace=False):
    from concourse.bass_utils import run_bass_kernel_spmd

    nc = _get_nc()
    in_maps = shard_inputs(x, W_in, W_fwd, W_bwd, W_out)
    res = run_bass_kernel_spmd(
        nc, in_maps, core_ids=list(range(N_CORES)), trace=trace
    )
    return gather_outputs(res.results), res


def kernel(x, W_in, W_fwd, W_bwd, W_out, gamma, beta):
    y, _ = run_on_hw(x, W_in, W_fwd, W_bwd, W_out)
    gamma = np.asarray(gamma, np.float32)
    beta = np.asarray(beta, np.float32)
    if not (np.all(gamma == 1.0) and np.all(beta == 0.0)):
        y = y * gamma + beta
    return y.astype(np.float32)


# revision 42
# speedup vs baseline: 1.0491x; 1.0491x over previous
"""Bidirectional simplified SSM kernel for Trainium2 (8 NeuronCores).

Math (per batch element b):
    z = x @ W_in                                  [L, DI]
    fwd:  o = z @ W_fwd; delta = sigmoid(o[:, :DI]); gate = o[:, DI:] * z
          h_t = delta_t * h_{t-1} + gate_t        (t ascending)
    bwd:  same with W_bwd, t descending
    y    = concat(h_fwd, h_bwd) @ W_out + x
    out  = LayerNorm(y) * gamma + beta

Sharding: 8 cores = 4 batches x 2 sequence halves, each with a 64-token
halo (delta ~ sigmoid(small) ~ 0.5 forgets cross-boundary state to
~1e-19 over 64 steps; no cross-core communication).

Precision/layout plan (rel err ~1.3e-2 vs the 2e-2 gate):
 - Host ships x twice: natural fp16 (residual/LN) and pre-transposed
   fp8 (z GEMM rhs), plus weights pre-packed in exact SBUF layout.
 - z GEMM: fp8 DoubleRow (2 K-tiles/instr, 0.5 cycles/row) with a
   split-W_in correction pass (W_in8 + fp8(W_in - W_in8)).
 - o GEMM: fp16 over the fp16 z (kills the z8/W quantization terms).
 - out GEMM: fp8 DoubleRow over the fp8 scan output h, with a split
   W_out correction pass.
 - Row sums of x for the LayerNorm mean ship precomputed from the host.

Engine plan: PE z/o/out GEMMs; ACT sigmoids + PSUM->SBUF converts +
copy-with-rowsum + half the squares + sqrt; DVE gates, all four scan
chains, residual add (fp16 2x), the other squares via mult + running-
sum scan, normalize (fp16 4x tensor_scalar); GPSIMD stats smalls and
most normalizes.  All input DMAs issue from SP HWDGE in priority order
(the transposed x streams through a 2-buffer rotation); y chunks DMA
out per-chunk as their normalize completes, middle-out, software-
pipelined with a 3-stage lag so no queue head-of-line blocks.

Hardware-validity notes learned the hard way: GPSIMD cannot access
PSUM or run scan/stt/divide ops; tensor_tensor_reduce crashes the
runtime; PSUM-draining reads must cover whole accumulation groups.
"""

import os
import sys

for _p in ("/opt/trn_rl_repo", "/root/.axon_site/_ro/trn_rl_repo"):
    if os.path.isdir(_p) and _p not in sys.path:
        sys.path.insert(0, _p)

import ml_dtypes
import numpy as np

import concourse.bacc as bacc
import concourse.bass as bass
import concourse.mybir as mybir
import concourse.tile as tile

P = 128
LN_EPS = 1e-5

B, L, D, DI = 4, 4096, 2048, 256
HALO = 64
T_CORE = L // 2            # tokens owned per core
T_CTX = T_CORE + 2 * HALO  # context tokens incl. halo
T_SCAN = T_CORE + HALO     # tokens each direction scans over
N_CORES = 8

F8 = ml_dtypes.float8_e4m3
DR = mybir.MatmulPerfMode.DoubleRow

# interleaved so both scan directions get their first segment early
SEG_ORDER = [0, 4, 1, 3, 2]
# middle-out: middle chunks' h_fwd/h_bwd complete first
CHUNK_ORDER = [9, 10, 8, 11, 7, 12, 6, 13, 5, 14, 4, 15, 3, 2, 1, 0]


def build_nc():
    d, di = D, DI
    kd = d // P            # 16 K-blocks for the z GEMM
    ki = di // P           # 2  channel groups of DI
    mi2 = 2 * di // P      # 4  output channel groups of the o GEMM
    ncho = T_CORE // P     # 16 owned output chunks
    segs = [(s, min(512, T_CTX - s)) for s in range(0, T_CTX, 512)]
    ssegs = [(s, min(512, T_SCAN - s)) for s in range(0, T_SCAN, 512)]
    nseg = len(segs)
    assert nseg == len(ssegs) == len(SEG_ORDER)

    f8 = mybir.dt.float8e4
    f16 = mybir.dt.float16
    f32 = mybir.dt.float32
    AO = mybir.AluOpType
    AF = mybir.ActivationFunctionType

    nc = bacc.Bacc("TRN2", target_bir_lowering=False, debug=False)
    xth_d = nc.dram_tensor("xT8h", [P, kd, T_CTX], f8, kind="ExternalInput").ap()
    x_d = nc.dram_tensor("x16", [T_CORE, d], f16, kind="ExternalInput").ap()
    win_d = nc.dram_tensor("W_in8", [P, kd, di], f8, kind="ExternalInput").ap()
    winr_d = nc.dram_tensor("W_in8r", [P, kd, di], f8, kind="ExternalInput").ap()
    wf_d = nc.dram_tensor("W_fwd16", [P, ki, 2 * di], f16, kind="ExternalInput").ap()
    wb_d = nc.dram_tensor("W_bwd16", [P, ki, 2 * di], f16, kind="ExternalInput").ap()
    wo_d = nc.dram_tensor("W_out8", [P, mi2, d], f8, kind="ExternalInput").ap()
    wor_d = nc.dram_tensor("W_out8r", [P, mi2, d], f8, kind="ExternalInput").ap()
    sx_d = nc.dram_tensor("sx", [P, T_CORE // P], f32, kind="ExternalInput").ap()
    y_d = nc.dram_tensor("y", [T_CORE, d], f16, kind="ExternalOutput").ap()

    inv_d = 1.0 / d

    with tile.TileContext(nc) as tc:
        with (
            tc.tile_pool(name="const", bufs=1) as cpool,
            tc.tile_pool(name="xt", bufs=1) as xtpool,
            tc.tile_pool(name="xn", bufs=1) as xnpool,
            tc.tile_pool(name="z", bufs=1) as zpool,
            tc.tile_pool(name="dg", bufs=1) as dgpool,
            tc.tile_pool(name="y16", bufs=3) as ypool,
            tc.tile_pool(name="ssm", bufs=2) as spool,
            tc.tile_pool(name="sc", bufs=1) as scpool,
            tc.tile_pool(name="sq", bufs=1) as sqpool,
            tc.tile_pool(name="yo", bufs=2) as yopool,
            tc.tile_pool(name="st", bufs=4) as stpool,
            tc.tile_pool(name="ps", bufs=4, space="PSUM") as pspool,
        ):
            # ---- pool-issued input DMAs, priority order ----
            w_in8 = cpool.tile([P, kd, di], f8)
            w_in8r = cpool.tile([P, kd, di], f8)
            w_f16 = cpool.tile([P, ki, 2 * di], f16)
            w_b16 = cpool.tile([P, ki, 2 * di], f16)
            w_o8 = cpool.tile([P, mi2, d], f8)
            w_o8r = cpool.tile([P, mi2, d], f8)
            x16 = xnpool.tile([P, ncho, d], f16)
            # x^T streams through a 2-buffer rotation (not resident)
            xth = [xtpool.tile([P, kd, 512], f8, name=f"xth{i}")
                   for i in range(2)]

            eps_t = cpool.tile([P, 1], f32)
            nc.gpsimd.memset(eps_t[:], LN_EPS)
            sx = cpool.tile([P, ncho], f32)
            nc.sync.dma_start(sx[:], sx_d)
            ones16 = cpool.tile([P, d], f16)
            nc.gpsimd.memset(ones16[:], 1.0)

            # all input DMAs on SP HWDGE (SEQ frees before the transfer, and
            # the pool queue stays clear for gate/scan work); transfer order
            # on the DMA engines = issue order = priority order
            def x16_quad(q):
                nc.sync.dma_start(
                    x16[:, 4 * q:4 * q + 4, :],
                    x_d[512 * q:512 * (q + 1), :].rearrange(
                        "(c p) d -> p c d", p=P
                    ),
                )

            def xt8_seg(k):
                si = SEG_ORDER[k]
                s0, ssz = segs[si]
                nc.sync.dma_start(
                    xth[k % 2][:, :, :ssz], xth_d[:, :, s0:s0 + ssz]
                )

            # only xth[1]'s tail is ever read beyond its DMA'd width (the
            # 128-token segment 4 reads the full 512); disjoint from the DMA
            # region so the transfer is not delayed
            nc.gpsimd.memset(xth[1][:, :, 128:], 0.0)
            nc.sync.dma_start(w_in8[:], win_d)
            nc.sync.dma_start(w_in8r[:], winr_d)
            xt8_seg(0)
            xt8_seg(1)
            nc.sync.dma_start(w_f16[:], wf_d)
            nc.sync.dma_start(w_b16[:], wb_d)


            # ---- z GEMM (double-fp8 DoubleRow) + o GEMMs, seg-interleaved ----
            # widths padded so every PSUM-draining op covers the full 512
            # columns of its PSUM tile (partial reads would leave a WAR gap
            # against the next accumulation group on the same bank)
            zw = (T_CTX // 512 + 1) * 512 + HALO          # 2624
            dgw = (T_SCAN // 512 + 1) * 512               # 2560
            z16 = zpool.tile([P, ki, zw], f16)
            d_f = dgpool.tile([P, ki, dgw], f16)
            g_f = dgpool.tile([P, ki, dgw], f16)
            h_f = dgpool.tile([P, ki, T_SCAN], f8)
            d_b = dgpool.tile([P, ki, dgw], f16)
            g_b = dgpool.tile([P, ki, dgw], f16)
            h_b = dgpool.tile([P, ki, T_SCAN], f8)

            def z_seg(k):
                si = SEG_ORDER[k]
                s0, ssz = segs[si]
                xh = xth[k % 2]
                pz = pspool.tile([P, 1024], f32, tag="ps", name="pz")
                for m in range(ki):
                    pv = pz[:, m * 512:(m + 1) * 512]
                    passes = [(w_in8, xh), (w_in8r, xh)]
                    for pi, (w8, xs) in enumerate(passes):
                        for k8 in range(kd // 2):
                            nc.tensor.matmul(
                                pv,
                                w8[:, 2 * k8:2 * k8 + 2, m * P:(m + 1) * P],
                                xs[:, 2 * k8:2 * k8 + 2, :],
                                start=(pi == 0 and k8 == 0),
                                stop=(pi == 1 and k8 == kd // 2 - 1),
                                perf_mode=DR,
                            )
                # full-width 2D convert: depends on both accumulation groups
                nc.scalar.copy(z16[:, :, s0:s0 + 512], pz[:])

            def o_seg(si, reverse):
                s0, ssz = ssegs[si]
                tok_off = HALO if reverse else 0
                w16 = w_b16 if reverse else w_f16
                dt = d_b if reverse else d_f
                gt = g_b if reverse else g_f
                zsl = slice(tok_off + s0, tok_off + s0 + 512)
                # deltas and gates in separate PSUM tiles: the ACT sigmoids
                # drain poA fast; only poB waits on the gate engines
                poA = pspool.tile([P, 1024], f32, tag="ps", name="poA")
                poB = pspool.tile([P, 1024], f32, tag="ps", name="poB")
                for m2 in range(mi2):
                    po = poA if m2 < ki else poB
                    pv = po[:, (m2 % ki) * 512:(m2 % ki + 1) * 512]
                    for kb in range(ki):
                        nc.tensor.matmul(
                            pv,
                            w16[:, kb, m2 * P:(m2 + 1) * P],
                            z16[:, kb, zsl],
                            start=(kb == 0),
                            stop=(kb == ki - 1),
                        )
                # GPSIMD cannot touch PSUM, so sigmoids (ACT) and gates
                # (DVE) drain it fused with their real work, one 3D
                # instruction per segment each
                nc.scalar.activation(
                    dt[:, :, s0:s0 + 512], poA[:], AF.Sigmoid
                )
                nc.vector.tensor_tensor(
                    gt[:, :, s0:s0 + 512], poB[:],
                    z16[:, :, zsl], AO.mult,
                )

            def scan_seg(si, reverse):
                s0, ssz = ssegs[si]
                dt, gt, ht = (d_b, g_b, h_b) if reverse else (d_f, g_f, h_f)
                first = si == (len(ssegs) - 1 if reverse else 0)
                for kb in range(ki):
                    e = nc.vector
                    if not reverse:
                        init = 0.0 if first else ht[:, kb, s0 - 1:s0]
                        e.tensor_tensor_scan(
                            ht[:, kb, s0:s0 + ssz],
                            dt[:, kb, s0:s0 + ssz],
                            gt[:, kb, s0:s0 + ssz],
                            init,
                            AO.mult,
                            AO.add,
                        )
                    else:
                        hi = s0 + ssz
                        init = 0.0 if first else ht[:, kb, hi:hi + 1]
                        e.tensor_tensor_scan(
                            ht[:, kb, s0:s0 + ssz][:, ::-1],
                            dt[:, kb, s0:s0 + ssz][:, ::-1],
                            gt[:, kb, s0:s0 + ssz][:, ::-1],
                            init,
                            AO.mult,
                            AO.add,
                        )

            # PE/consumer order: z segs interleaved with o segs as the
            # transposed input lands; fwd o ascending, bwd o descending.
            # pad region read by the last bwd o-segment, never written
            nc.gpsimd.memset(z16[:, :, 5 * 512:], 0.0)
            z_seg(0)
            xt8_seg(2)
            z_seg(1)
            xt8_seg(3)
            o_seg(0, reverse=False)
            o_seg(nseg - 1, reverse=True)
            scan_seg(0, reverse=False)
            scan_seg(nseg - 1, reverse=True)
            fwd_i, bwd_i = 1, nseg - 2
            for k in range(2, nseg):
                z_seg(k)
                if k == 2:
                    xt8_seg(4)
                    x16_quad(2)
                    nc.sync.dma_start(w_o8[:], wo_d)
                if k == 3:
                    x16_quad(3)
                    nc.sync.dma_start(w_o8r[:], wor_d)
                if k == 4:
                    x16_quad(1)
                    x16_quad(0)
                if k % 2 == 0:
                    o_seg(fwd_i, reverse=False)
                    scan_seg(fwd_i, reverse=False)
                    fwd_i += 1
                else:
                    o_seg(bwd_i, reverse=True)
                    scan_seg(bwd_i, reverse=True)
                    bwd_i -= 1
            while fwd_i < nseg or bwd_i >= 0:
                if fwd_i < nseg:
                    o_seg(fwd_i, reverse=False)
                    scan_seg(fwd_i, reverse=False)
                    fwd_i += 1
                if bwd_i >= 0:
                    o_seg(bwd_i, reverse=True)
                    scan_seg(bwd_i, reverse=True)
                    bwd_i -= 1

            # ---- out GEMM + residual + LayerNorm per owned chunk ----
            # Four emission stages with 1-chunk lags so the in-order queues
            # never head-of-line block on the cross-engine stat chain.
            # tensor_tensor_reduce is broken in the HW runtime, so:
            #   A: PE out GEMM (fp8 DR, W_out hi+lo); ACT copy+accum
            #      (ssm16 + row-sum of the ssm part; sum(x) ships from host)
            #   B: DVE residual add (fp16 2x); sumsq via ACT Square+accum on
            #      ~half the chunks, else DVE mult + running-sum scan;
            #      GPSIMD mean stats
            #   C: GPSIMD var; ACT sqrt; DVE reciprocal
            #   D: normalize (DVE/GPSIMD tensor_scalar) + SP y DMA
            live = {}

            def chunk_a(oc):
                tb = HALO + oc * P     # context-token base of this chunk
                ssm16 = spool.tile([P, d], f16, name="ssm16")
                st = stpool.tile([P, 12], f32, name="st")
                pys = [pspool.tile([P, 1024], f32, tag="ps", name="py")
                       for _ in range(2)]
                for dgi in range(4):
                    dsl = slice(dgi * 512, (dgi + 1) * 512)
                    pv = pys[dgi // 2][:, (dgi % 2) * 512:(dgi % 2 + 1) * 512]
                    nc.tensor.matmul(
                        pv, h_f[:, :, tb:tb + P], w_o8[:, 0:2, dsl],
                        start=True, stop=False, perf_mode=DR,
                    )
                    nc.tensor.matmul(
                        pv, h_b[:, :, tb - HALO:tb - HALO + P],
                        w_o8[:, 2:4, dsl],
                        start=False, stop=False, perf_mode=DR,
                    )
                    nc.tensor.matmul(
                        pv, h_f[:, :, tb:tb + P], w_o8r[:, 0:2, dsl],
                        start=False, stop=False, perf_mode=DR,
                    )
                    nc.tensor.matmul(
                        pv, h_b[:, :, tb - HALO:tb - HALO + P],
                        w_o8r[:, 2:4, dsl],
                        start=False, stop=True, perf_mode=DR,
                    )
                for half in range(2):
                    hsl = slice(half * 1024, (half + 1) * 1024)
                    nc.scalar.activation(
                        ssm16[:, hsl], pys[half][:], AF.Copy,
                        accum_out=st[:, half:half + 1],
                    )
                live[oc] = (ssm16, st)

            def chunk_b(oc, idx):
                ssm16, st = live[oc]
                y16 = ypool.tile([P, d], f16, name="y16")
                nc.vector.tensor_tensor(
                    y16[:], ssm16[:], x16[:, oc, :], AO.add
                )
                if idx % 2 == 0:
                    # sumsq on DVE: fused square + row-sum in one op
                    sq = sqpool.tile([P, d], f16, name="sq")
                    nc.vector.scalar_tensor_tensor(
                        sq[:], y16[:], 1.0, y16[:], AO.mult, AO.mult,
                        accum_out=st[:, 2:3],
                    )
                    live[oc] = (y16, st, st[:, 2:3])
                else:
                    sq = sqpool.tile([P, d], f16, name="sq")
                    nc.scalar.activation(
                        sq[:], y16[:], AF.Square, accum_out=st[:, 2:3]
                    )
                    live[oc] = (y16, st, st[:, 2:3])
                # mean = (st0+st1+sum_x)/d
                nc.gpsimd.tensor_tensor(st[:, 3:4], st[:, 0:1], st[:, 1:2], AO.add)
                nc.gpsimd.tensor_tensor(
                    st[:, 10:11], st[:, 3:4], sx[:, oc:oc + 1], AO.add
                )
                nc.gpsimd.tensor_scalar(st[:, 4:5], st[:, 10:11], inv_d, None, AO.mult)
                nc.gpsimd.tensor_tensor(st[:, 5:6], st[:, 4:5], st[:, 4:5], AO.mult)

            def chunk_c(oc):
                y16, st, sumsq = live[oc]
                nc.gpsimd.tensor_scalar(st[:, 9:10], sumsq, inv_d, None, AO.mult)
                nc.gpsimd.tensor_tensor(st[:, 6:7], st[:, 9:10], st[:, 5:6], AO.subtract)
                nc.scalar.activation(st[:, 7:8], st[:, 6:7], AF.Sqrt, bias=eps_t[:])
                nc.vector.reciprocal(st[:, 8:9], st[:, 7:8])

            def chunk_d(oc, idx):
                y16, st, _ = live.pop(oc)
                yo = yopool.tile([P, d], f16, name="yo")
                # normalize: GPSIMD (idle after the scans) takes most chunks
                e = nc.gpsimd if idx % 8 < 5 else nc.vector
                e.tensor_scalar(
                    yo[:], y16[:], st[:, 4:5], st[:, 8:9], AO.subtract, AO.mult
                )
                nc.sync.dma_start(y_d[oc * P:(oc + 1) * P, :], yo[:])

            for idx in range(ncho + 3):
                if idx < ncho:
                    chunk_a(CHUNK_ORDER[idx])
                if 1 <= idx < ncho + 1:
                    chunk_b(CHUNK_ORDER[idx - 1], idx - 1)
                if 2 <= idx < ncho + 2:
                    chunk_c(CHUNK_ORDER[idx - 2])
                if idx >= 3:
                    chunk_d(CHUNK_ORDER[idx - 3], idx - 3)

    nc.compile()
    return nc


_NC_CACHE = {}


def _get_nc():
    if "nc" not in _NC_CACHE:
        _NC_CACHE["nc"] = build_nc()
    return _NC_CACHE["nc"]


def _pack_weights(W_in, W_fwd, W_bwd, W_out):
    """Rearrange [K, M] weights into SBUF layout [128, K//128, M]."""
    def pack(w, dt):
        k, m = w.shape
        return np.ascontiguousarray(
            w.reshape(k // P, P, m).transpose(1, 0, 2)
        ).astype(dt)

    W_in = np.asarray(W_in, np.float32)
    W_in8 = W_in.astype(F8)
    W_in8r = (W_in - W_in8.astype(np.float32)).astype(F8)
    W_out = np.asarray(W_out, np.float32)
    W_out8 = W_out.astype(F8)
    W_out8r = (W_out - W_out8.astype(np.float32)).astype(F8)
    return {
        "W_in8": pack(W_in8.astype(np.float32), F8),
        "W_in8r": pack(W_in8r.astype(np.float32), F8),
        "W_fwd16": pack(np.asarray(W_fwd, np.float32), np.float16),
        "W_bwd16": pack(np.asarray(W_bwd, np.float32), np.float16),
        "W_out8": pack(W_out8.astype(np.float32), F8),
        "W_out8r": pack(W_out8r.astype(np.float32), F8),
    }


def shard_inputs(x, W_in, W_fwd, W_bwd, W_out):
    """Full x [B, L, D] -> 8 per-core input dicts."""
    x16 = np.asarray(x, np.float32).astype(np.float16)
    xpad = np.zeros((B, L + 2 * HALO, D), np.float16)
    xpad[:, HALO:HALO + L] = x16
    wmaps = _pack_weights(W_in, W_fwd, W_bwd, W_out)
    in_maps = []
    for b in range(B):
        for h in range(2):
            ctx = xpad[b, h * T_CORE:h * T_CORE + T_CTX]      # [T_CTX, D]
            xT = np.ascontiguousarray(
                ctx.T.reshape(D // P, P, T_CTX).transpose(1, 0, 2)
            )                                                  # [128, kd, T_CTX]
            xT8h = xT.astype(F8)
            xnat = np.ascontiguousarray(ctx[HALO:HALO + T_CORE])
            sx = np.ascontiguousarray(
                xnat.astype(np.float32).sum(axis=1).reshape(T_CORE // P, P).T
            )
            in_maps.append({"xT8h": xT8h, "x16": xnat, "sx": sx, **wmaps})
    return in_maps


def gather_outputs(results):
    out = np.empty((B, L, D), np.float32)
    for b in range(B):
        for h in range(2):
            out[b, h * T_CORE:(h + 1) * T_CORE] = results[b * 2 + h]["y"]
    return out


def run_on_hw(x, W_in, W_fwd, W_bwd, W_out, trace=False):
    from concourse.bass_utils import run_bass_kernel_spmd

    nc = _get_nc()
    in_maps = shard_inputs(x, W_in, W_fwd, W_bwd, W_out)
    res = run_bass_kernel_spmd(
        nc, in_maps, core_ids=list(range(N_CORES)), trace=trace
    )
    return gather_outputs(res.results), res


def kernel(x, W_in, W_fwd, W_bwd, W_out, gamma, beta):
    y, _ = run_on_hw(x, W_in, W_fwd, W_bwd, W_out)
    gamma = np.asarray(gamma, np.float32)
    beta = np.asarray(beta, np.float32)
    if not (np.all(gamma == 1.0) and np.all(beta == 0.0)):
        y = y * gamma + beta
    return y.astype(np.float32)
